# revision 5
# baseline (speedup 1.0000x reference)
"""TRN2 Bass kernel for nn_Decoder (2-layer GRU decoder, B=64, H=1024,
V=32000, T=32 greedy decode steps).

Strategy (8 NeuronCores, SPMD):
  - Tensor-parallel GRU: gate dim (3H=3072) sharded 8 ways -> each core
    computes a 128-wide slice of the hidden state per layer, then the
    h-vectors are AllGathered (transposed layout, [1024, 64]) each step.
  - Layer-1 input gates come from a host-precomputed table
    G = relu(emb) @ w_ih[0].T + b_ih[0]  (per-core gate slice, [32000, 384])
    gathered by token id with dma_gather -> no embedding matmul on device.
  - Output projection: vocab sharded 8 ways (4000/core), fp32 matmuls
    col-tiled across two PE column groups ([0:64] vocabA, [64:128] vocabB).
  - Greedy argmax: per-chunk top-1 via DVE max/max_index, exact
    first-occurrence tie-break via the (max-val)*1e12+idx reduce-min trick,
    cross-core combine via a tiny AllGather of (val, idx) candidates.

All matmul math is native fp32 (PE 2-pass) to keep the token trajectory
bit-faithful to the fp32 reference; any argmax flip would corrupt all
later logits.
"""

import sys

for p in ("/opt/trn_rl_repo", "/root/.axon_site"):
    if p not in sys.path:
        sys.path.insert(0, p)

import numpy as np

import concourse.bacc as bacc
import concourse.mybir as mybir
import concourse.tile as tile
from concourse.bass_utils import run_bass_kernel_spmd

F32 = mybir.dt.float32
I32 = mybir.dt.int32
I16 = mybir.dt.int16
U32 = mybir.dt.uint32

NCORES = 8
B = 64          # batch
H = 1024        # hidden/embed dim
KT = H // 128   # K tiles
GS = 3 * H // NCORES      # per-core gate slice = 384
HS = H // NCORES          # per-core hidden slice = 128
V = 32000
VS = V // NCORES          # per-core vocab slice = 4000
VH = VS // 2              # per col-group half = 2000
NCHUNK = 4                # 500-wide psum chunks per half
CW = VH // NCHUNK         # 500
HUGE = 1.0e12

AX = mybir.AxisListType
ALU = mybir.AluOpType
ACTF = mybir.ActivationFunctionType


def build(T: int):
    nc = bacc.Bacc("TRN2", target_bir_lowering=False, debug=False,
                   num_devices=NCORES)

    def din(name, shape):
        return nc.dram_tensor(name, shape, F32, kind="ExternalInput").ap()

    g_tab = din("g_tab", [V, GS])
    wh1T = din("wh1T", [H, GS])
    wi2T = din("wi2T", [H, GS])
    wh2T = din("wh2T", [H, GS])
    bh1 = din("bh1", [1, GS])
    bi2 = din("bi2", [1, GS])
    bh2 = din("bh2", [1, GS])
    outwT = din("outwT", [H, VS])
    outb = din("outb", [1, VS])
    h0T = din("h0T", [2, H, B])          # transposed initial hidden
    h0s = din("h0s", [2, B, HS])         # per-core hidden slices
    ident = din("ident", [B, B])
    cbase = din("cbase", [128, NCHUNK])   # chunk/col-group vocab base

    logits_out = nc.dram_tensor("logits_out", [T, B, VS], F32,
                                kind="ExternalOutput").ap()
    hfin = nc.dram_tensor("hfin", [2, 128, KT, B], F32,
                          kind="ExternalOutput").ap()

    rg = [list(range(NCORES))]

    with tile.TileContext(nc) as tc:
        with (
            tc.tile_pool(name="const", bufs=1) as cpool,
            tc.tile_pool(name="state", bufs=2) as spool,
            tc.tile_pool(name="work", bufs=1) as wpool,
            tc.tile_pool(name="pg", bufs=1, space="PSUM") as pg,
            tc.tile_pool(name="pp", bufs=4, space="PSUM") as pp,
            tc.tile_pool(name="dram", bufs=2, space="DRAM") as dpool,
        ):
            # ---------- constants / weights in SBUF ----------
            wh1T_sb = cpool.tile([128, KT, GS], F32, tag="wh1T")
            wi2T_sb = cpool.tile([128, KT, GS], F32, tag="wi2T")
            wh2T_sb = cpool.tile([128, KT, GS], F32, tag="wh2T")
            outwT_sb = cpool.tile([128, KT, VS], F32, tag="outwT")
            bh1_sb = cpool.tile([1, GS], F32, tag="bh1")
            bi2_sb = cpool.tile([1, GS], F32, tag="bi2")
            bh2_sb = cpool.tile([1, GS], F32, tag="bh2")
            outb_sb = cpool.tile([1, VS], F32, tag="outb")
            id_sb = cpool.tile([B, B], F32, tag="id")
            ones_sb = cpool.tile([1, B], F32, tag="ones")
            cbase_sb = cpool.tile([128, NCHUNK], F32, tag="cbase")

            for dst, src in ((wh1T_sb, wh1T), (wi2T_sb, wi2T),
                             (wh2T_sb, wh2T), (outwT_sb, outwT)):
                nc.sync.dma_start(
                    out=dst[:], in_=src.rearrange("(kt p) n -> p kt n", p=128))
            for dst, src in ((bh1_sb, bh1), (bi2_sb, bi2), (bh2_sb, bh2),
                             (outb_sb, outb), (id_sb, ident),
                             (cbase_sb, cbase)):
                nc.sync.dma_start(out=dst[:], in_=src)
            nc.vector.memset(ones_sb[:], 1.0)

            # ---------- state ----------
            hT = [spool.tile([128, KT, B], F32, tag=f"hT{l}", name=f"hT{l}")
                  for l in (0, 1)]
            hsl = [spool.tile([B, HS], F32, tag=f"hsl{l}", name=f"hsl{l}")
                   for l in (0, 1)]
            for l in (0, 1):
                nc.sync.dma_start(
                    out=hT[l][:], in_=h0T[l].rearrange("(kt p) b -> p kt b", p=128))
                nc.sync.dma_start(out=hsl[l][:], in_=h0s[l])
            idx16 = cpool.tile([16, 4], I16, tag="idx16")
            nc.vector.memset(idx16[:], 0)   # START_TOKEN = 0

            amax_out_prev = None

            for t in range(T):
                # ---- early (off critical path): gh matmuls from h(t-1) ----
                ps_gh1 = pg.tile([B, GS], F32, tag="gh1")
                for k in range(KT):
                    nc.tensor.matmul(ps_gh1[:], hT[0][:, k, :], wh1T_sb[:, k, :],
                                     start=(k == 0), stop=False)
                nc.tensor.matmul(ps_gh1[:], ones_sb[:], bh1_sb[:],
                                 start=False, stop=True)
                ps_gh2 = pg.tile([B, GS], F32, tag="gh2")
                for k in range(KT):
                    nc.tensor.matmul(ps_gh2[:], hT[1][:, k, :], wh2T_sb[:, k, :],
                                     start=(k == 0), stop=False)
                nc.tensor.matmul(ps_gh2[:], ones_sb[:], bh2_sb[:],
                                 start=False, stop=True)

                # ---- token combine from previous step's candidate AG ----
                if t > 0:
                    cand = wpool.tile([16, 8, 4, 2], F32, tag="cand")
                    nc.sync.dma_start(
                        out=cand[:],
                        in_=amax_out_prev[:].rearrange(
                            "c (j p) v -> p c j v", p=16))
                    maxv = wpool.tile([16, 4], F32, tag="maxv")
                    nc.vector.tensor_reduce(
                        out=maxv[:],
                        in_=cand[:, :, :, 0].rearrange("p c j -> p j c"),
                        axis=AX.X, op=ALU.max)
                    comb = wpool.tile([16, 4, 8], F32, tag="comb")
                    for j in range(4):
                        nc.vector.tensor_scalar(
                            out=comb[:, j, :], in0=cand[:, :, j, 0],
                            scalar1=maxv[:, j:j + 1], scalar2=-HUGE,
                            op0=ALU.subtract, op1=ALU.mult)
                        nc.vector.tensor_tensor(
                            out=comb[:, j, :], in0=comb[:, j, :],
                            in1=cand[:, :, j, 1], op=ALU.add)
                    winner = wpool.tile([16, 4], F32, tag="winner")
                    nc.vector.tensor_reduce(out=winner[:], in_=comb[:],
                                            axis=AX.X, op=ALU.min)
                    win_i = wpool.tile([16, 4], I32, tag="win_i")
                    nc.vector.tensor_copy(win_i[:], winner[:])
                    nc.vector.tensor_copy(idx16[:], win_i[:])

                # ---- layer-1 input gates via token gather ----
                gi1 = wpool.tile([128, 1, GS], F32, tag="gi1")
                nc.gpsimd.dma_gather(
                    out_ap=gi1[:], in_ap=g_tab, idxs_ap=idx16[:],
                    num_idxs=B, num_idxs_reg=B, elem_size=GS)

                # ---- GRU cells ----
                def cell(l, gi_ap, gh_ps):
                    rz_pre = wpool.tile([B, 256], F32, tag="rz_pre")
                    nc.vector.tensor_tensor(out=rz_pre[:], in0=gi_ap[:, 0:256],
                                            in1=gh_ps[:, 0:256], op=ALU.add)
                    rz = wpool.tile([B, 256], F32, tag="rz")
                    nc.scalar.activation(rz[:], rz_pre[:], ACTF.Sigmoid)
                    n_pre = wpool.tile([B, HS], F32, tag="n_pre")
                    nc.vector.tensor_tensor(out=n_pre[:], in0=rz[:, 0:128],
                                            in1=gh_ps[:, 256:384], op=ALU.mult)
                    nc.vector.tensor_tensor(out=n_pre[:], in0=n_pre[:],
                                            in1=gi_ap[:, 256:384], op=ALU.add)
                    n_t = wpool.tile([B, HS], F32, tag="n_t")
                    nc.scalar.activation(n_t[:], n_pre[:], ACTF.Tanh)
                    d_t = wpool.tile([B, HS], F32, tag="d_t")
                    nc.vector.tensor_tensor(out=d_t[:], in0=hsl[l][:],
                                            in1=n_t[:], op=ALU.subtract)
                    h_new = spool.tile([B, HS], F32, tag=f"hsl{l}", name=f"hnew{l}")
                    nc.vector.tensor_tensor(out=d_t[:], in0=rz[:, 128:256],
                                            in1=d_t[:], op=ALU.mult)
                    nc.vector.tensor_tensor(out=h_new[:], in0=d_t[:],
                                            in1=n_t[:], op=ALU.add)
                    return h_new

                def exchange(l, h_new):
                    # transpose [B, HS] -> [HS=128, B], AllGather to [H, B]
                    tr_ps = pg.tile([128, B], F32, tag="tr")
                    nc.tensor.transpose(tr_ps[:], h_new[:], id_sb[:])
                    hT_sl = wpool.tile([128, B], F32, tag="hT_sl")
                    nc.scalar.copy(hT_sl[:], tr_ps[:])
                    ag_in = dpool.tile([128, B], F32, tag=f"agin{l}", name=f"agin{l}")
                    ag_out = dpool.tile([NCORES, 128, B], F32, tag=f"agout{l}", name=f"agout{l}")
                    nc.sync.dma_start(out=ag_in[:], in_=hT_sl[:])
                    nc.gpsimd.collective_compute(
                        "AllGather", ALU.bypass, replica_groups=rg,
                        ins=[ag_in.opt()], outs=[ag_out.opt()])
                    hT_new = spool.tile([128, KT, B], F32, tag=f"hT{l}", name=f"hTn{l}")
                    nc.sync.dma_start(
                        out=hT_new[:], in_=ag_out[:].rearrange("r p b -> p r b"))
                    return hT_new

                h1_new = cell(0, gi1[0:B, 0, :], ps_gh1)
                hT1_new = exchange(0, h1_new)

                ps_gi2 = pg.tile([B, GS], F32, tag="gi2")
                for k in range(KT):
                    nc.tensor.matmul(ps_gi2[:], hT1_new[:, k, :], wi2T_sb[:, k, :],
                                     start=(k == 0), stop=False)
                nc.tensor.matmul(ps_gi2[:], ones_sb[:], bi2_sb[:],
                                 start=False, stop=True)

                gi2_sb = wpool.tile([B, GS], F32, tag="gi2_sb")
                nc.scalar.copy(gi2_sb[:], ps_gi2[:])
                h2_new = cell(1, gi2_sb, ps_gh2)
                hT2_new = exchange(1, h2_new)

                # ---- projection (vocab-sharded, col-tiled x2) ----
                logits_sb = wpool.tile([128, VH], F32, tag="logits")
                cv8 = wpool.tile([128, NCHUNK, 8], F32, tag="cv8")
                ci8 = wpool.tile([128, NCHUNK, 8], U32, tag="ci8")
                for c in range(NCHUNK):
                    ps_p = pp.tile([128, 512], F32, tag="proj")
                    lo = c * CW
                    for k in range(KT):
                        nc.tensor.matmul(
                            ps_p[0:B, 0:CW], hT2_new[:, k, :],
                            outwT_sb[:, k, lo:lo + CW],
                            start=(k == 0), stop=False, tile_position=(0, 0))
                        nc.tensor.matmul(
                            ps_p[B:128, 0:CW], hT2_new[:, k, :],
                            outwT_sb[:, k, VH + lo:VH + lo + CW],
                            start=(k == 0), stop=False, tile_position=(0, 64))
                    nc.tensor.matmul(ps_p[0:B, 0:CW], ones_sb[:],
                                     outb_sb[:, lo:lo + CW],
                                     start=False, stop=True, tile_position=(0, 0))
                    nc.tensor.matmul(ps_p[B:128, 0:CW], ones_sb[:],
                                     outb_sb[:, VH + lo:VH + lo + CW],
                                     start=False, stop=True, tile_position=(0, 64))
                    nc.scalar.copy(logits_sb[:, lo:lo + CW], ps_p[:, 0:CW])
                    nc.vector.max(cv8[:, c, :], logits_sb[:, lo:lo + CW])
                    nc.vector.max_index(ci8[:, c, :], cv8[:, c, :],
                                        logits_sb[:, lo:lo + CW])

                # logits out (two halves: psum rows 0:64 vocabA, 64:128 vocabB)
                nc.sync.dma_start(out=logits_out[t][:, 0:VH],
                                  in_=logits_sb[0:B, :])
                nc.sync.dma_start(out=logits_out[t][:, VH:VS],
                                  in_=logits_sb[B:128, :])

                # ---- local argmax combine ----
                gidx = wpool.tile([128, NCHUNK], F32, tag="gidx")
                nc.vector.tensor_copy(gidx[:], ci8[:, :, 0])
                nc.vector.tensor_tensor(out=gidx[:], in0=gidx[:],
                                        in1=cbase_sb[:], op=ALU.add)
                mx4 = wpool.tile([128, 1], F32, tag="mx4")
                nc.vector.tensor_reduce(out=mx4[:], in_=cv8[:, :, 0],
                                        axis=AX.X, op=ALU.max)
                cmb = wpool.tile([128, NCHUNK], F32, tag="cmb")
                nc.vector.tensor_scalar(out=cmb[:], in0=cv8[:, :, 0],
                                        scalar1=mx4[:], scalar2=-HUGE,
                                        op0=ALU.subtract, op1=ALU.mult)
                nc.vector.tensor_tensor(out=cmb[:], in0=cmb[:], in1=gidx[:],
                                        op=ALU.add)
                bidx = wpool.tile([128, 1], F32, tag="bidx")
                nc.vector.tensor_reduce(out=bidx[:], in_=cmb[:],
                                        axis=AX.X, op=ALU.min)
                # fold col-group halves: shuffle partitions 64:128 -> 0:64
                shv = wpool.tile([128, 1], F32, tag="shv")
                shi = wpool.tile([128, 1], F32, tag="shi")
                mask = [16 + (g % 16) for g in range(32)]
                nc.vector.stream_shuffle(shv[:], mx4[:], mask)
                nc.vector.stream_shuffle(shi[:], bidx[:], mask)
                pack = wpool.tile([B, 2], F32, tag="pack")
                nc.vector.tensor_tensor(out=pack[:, 0:1], in0=mx4[0:B, :],
                                        in1=shv[0:B, :], op=ALU.max)
                c2a = wpool.tile([B, 1], F32, tag="c2a")
                c2b = wpool.tile([B, 1], F32, tag="c2b")
                nc.vector.tensor_scalar(out=c2a[:], in0=mx4[0:B, :],
                                        scalar1=pack[:, 0:1], scalar2=-HUGE,
                                        op0=ALU.subtract, op1=ALU.mult)
                nc.vector.tensor_tensor(out=c2a[:], in0=c2a[:], in1=bidx[0:B, :],
                                        op=ALU.add)
                nc.vector.tensor_scalar(out=c2b[:], in0=shv[0:B, :],
                                        scalar1=pack[:, 0:1], scalar2=-HUGE,
                                        op0=ALU.subtract, op1=ALU.mult)
                nc.vector.tensor_tensor(out=c2b[:], in0=c2b[:], in1=shi[0:B, :],
                                        op=ALU.add)
                nc.vector.tensor_tensor(out=pack[:, 1:2], in0=c2a[:], in1=c2b[:],
                                        op=ALU.min)

                am_in = dpool.tile([B, 2], F32, tag="amin")
                am_out = dpool.tile([NCORES, B, 2], F32, tag="amout")
                nc.sync.dma_start(out=am_in[:], in_=pack[:])
                nc.gpsimd.collective_compute(
                    "AllGather", ALU.bypass, replica_groups=rg,
                    ins=[am_in.opt()], outs=[am_out.opt()])
                amax_out_prev = am_out

                hT[0], hT[1] = hT1_new, hT2_new
                hsl[0], hsl[1] = h1_new, h2_new

            for l in (0, 1):
                nc.sync.dma_start(out=hfin[l], in_=hT[l][:])
    nc.compile()
    return nc


# ----------------------------------------------------------------------
# host side
# ----------------------------------------------------------------------

def prep(inputs):
    emb = np.asarray(inputs["emb"], np.float32)
    w_ih = np.asarray(inputs["w_ih"], np.float32)
    w_hh = np.asarray(inputs["w_hh"], np.float32)
    b_ih = np.asarray(inputs["b_ih"], np.float32)
    b_hh = np.asarray(inputs["b_hh"], np.float32)
    out_w = np.asarray(inputs["out_w"], np.float32)
    out_b = np.asarray(inputs["out_b"], np.float32)
    h0 = np.asarray(inputs["encoder_hidden"], np.float32)  # [2, B, H]

    emb_relu = np.maximum(emb, 0.0)
    G = emb_relu @ w_ih[0].T + b_ih[0]          # [V, 3H]
    outwT = np.ascontiguousarray(out_w.T)        # [H, V]
    h0T = np.ascontiguousarray(np.transpose(h0, (0, 2, 1)))  # [2, H, B]
    ident = np.eye(B, dtype=np.float32)

    in_maps = []
    for c in range(NCORES):
        gs = np.r_[c * HS:(c + 1) * HS,
                   H + c * HS:H + (c + 1) * HS,
                   2 * H + c * HS:2 * H + (c + 1) * HS]
        cbase = np.zeros((128, NCHUNK), np.float32)
        cbase[:, :] = (np.arange(NCHUNK) * CW)[None, :]
        cbase[B:, :] += VH
        cbase += c * VS
        in_maps.append({
            "g_tab": np.ascontiguousarray(G[:, gs]),
            "wh1T": np.ascontiguousarray(w_hh[0].T[:, gs]),
            "wi2T": np.ascontiguousarray(w_ih[1].T[:, gs]),
            "wh2T": np.ascontiguousarray(w_hh[1].T[:, gs]),
            "bh1": np.ascontiguousarray(b_hh[0][gs][None, :]),
            "bi2": np.ascontiguousarray(b_ih[1][gs][None, :]),
            "bh2": np.ascontiguousarray(b_hh[1][gs][None, :]),
            "outwT": np.ascontiguousarray(outwT[:, c * VS:(c + 1) * VS]),
            "outb": np.ascontiguousarray(out_b[c * VS:(c + 1) * VS][None, :]),
            "h0T": h0T,
            "h0s": np.ascontiguousarray(h0[:, :, c * HS:(c + 1) * HS]),
            "ident": ident,
            "cbase": cbase,
        })
    return in_maps


_CACHE = {}


def run(inputs, trace=False):
    T = int(np.asarray(inputs.get("max_length", 32)))
    if T not in _CACHE:
        _CACHE[T] = build(T)
    nc = _CACHE[T]
    in_maps = prep(inputs)
    res = run_bass_kernel_spmd(nc, in_maps, core_ids=list(range(NCORES)),
                               trace=trace)
    # assemble full outputs
    logits = np.concatenate(
        [res.results[c]["logits_out"] for c in range(NCORES)], axis=2)
    decoder_outputs = np.ascontiguousarray(np.transpose(logits, (1, 0, 2)))
    hf = res.results[0]["hfin"]  # [2, 128, KT, B]
    h_final = np.ascontiguousarray(
        np.transpose(hf.reshape(2, 128, KT, B), (0, 3, 2, 1))
        .reshape(2, B, H))
    return (decoder_outputs, h_final), res


def kernel(**inputs):
    (decoder_outputs, h_final), _ = run(inputs, trace=False)
    return decoder_outputs, h_final


# revision 6
# speedup vs baseline: 1.0809x; 1.0809x over previous
"""TRN2 Bass kernel for nn_Decoder (2-layer GRU decoder, B=64, H=1024,
V=32000, T=32 greedy decode steps).

Strategy (8 NeuronCores, SPMD):
  - Tensor-parallel GRU: gate dim (3H=3072) sharded 8 ways -> each core
    computes a 128-wide slice of the hidden state per layer, then the
    h-vectors are AllGathered (transposed layout, [1024, 64]) each step.
  - Layer-1 input gates come from a host-precomputed table
    G = relu(emb) @ w_ih[0].T + b_ih[0]  (per-core gate slice, [32000, 384])
    gathered by token id with dma_gather -> no embedding matmul on device.
  - Output projection: vocab sharded 8 ways (4000/core), fp32 matmuls
    col-tiled across two PE column groups ([0:64] vocabA, [64:128] vocabB).
  - Greedy argmax: per-chunk top-1 via DVE max/max_index, exact
    first-occurrence tie-break via the (max-val)*1e12+idx reduce-min trick,
    cross-core combine via a tiny AllGather of (val, idx) candidates.

All matmul math is native fp32 (PE 2-pass) to keep the token trajectory
bit-faithful to the fp32 reference; any argmax flip would corrupt all
later logits.
"""

import sys

for p in ("/opt/trn_rl_repo", "/root/.axon_site"):
    if p not in sys.path:
        sys.path.insert(0, p)

import numpy as np

import concourse.bacc as bacc
import concourse.mybir as mybir
import concourse.tile as tile
from concourse.bass_utils import run_bass_kernel_spmd

F32 = mybir.dt.float32
I32 = mybir.dt.int32
I16 = mybir.dt.int16
U32 = mybir.dt.uint32

NCORES = 8
B = 64          # batch
H = 1024        # hidden/embed dim
KT = H // 128   # K tiles
GS = 3 * H // NCORES      # per-core gate slice = 384
HS = H // NCORES          # per-core hidden slice = 128
V = 32000
VS = V // NCORES          # per-core vocab slice = 4000
VH = VS // 2              # per col-group half = 2000
NCHUNK = 4                # 500-wide psum chunks per half
CW = VH // NCHUNK         # 500
HUGE = 1.0e12

AX = mybir.AxisListType
ALU = mybir.AluOpType
ACTF = mybir.ActivationFunctionType


def build(T: int):
    nc = bacc.Bacc("TRN2", target_bir_lowering=False, debug=False,
                   num_devices=NCORES)

    def din(name, shape):
        return nc.dram_tensor(name, shape, F32, kind="ExternalInput").ap()

    g_tab = din("g_tab", [V, GS])
    wh1T = din("wh1T", [H, GS])
    wi2T = din("wi2T", [H, GS])
    wh2T = din("wh2T", [H, GS])
    bh1 = din("bh1", [1, GS])
    bi2 = din("bi2", [1, GS])
    bh2 = din("bh2", [1, GS])
    outwT = din("outwT", [H, VS])
    outb = din("outb", [1, VS])
    h0T = din("h0T", [2, H, B])          # transposed initial hidden
    h0s = din("h0s", [2, B, HS])         # per-core hidden slices
    ident = din("ident", [B, B])
    cbase = din("cbase", [128, NCHUNK])   # chunk/col-group vocab base

    logits_out = nc.dram_tensor("logits_out", [T, B, VS], F32,
                                kind="ExternalOutput").ap()
    hfin = nc.dram_tensor("hfin", [2, 128, KT, B], F32,
                          kind="ExternalOutput").ap()

    rg = [list(range(NCORES))]

    with tile.TileContext(nc) as tc:
        with (
            tc.tile_pool(name="const", bufs=1) as cpool,
            tc.tile_pool(name="state", bufs=2) as spool,
            tc.tile_pool(name="work", bufs=1) as wpool,
            tc.tile_pool(name="pg", bufs=1, space="PSUM") as pg,
            tc.tile_pool(name="pp", bufs=4, space="PSUM") as pp,
            tc.tile_pool(name="dram", bufs=2, space="DRAM") as dpool,
        ):
            # ---------- constants / weights in SBUF ----------
            wh1T_sb = cpool.tile([128, KT, GS], F32, tag="wh1T")
            wi2T_sb = cpool.tile([128, KT, GS], F32, tag="wi2T")
            wh2T_sb = cpool.tile([128, KT, GS], F32, tag="wh2T")
            outwT_sb = cpool.tile([128, KT, VS], F32, tag="outwT")
            bh1_sb = cpool.tile([1, GS], F32, tag="bh1")
            bi2_sb = cpool.tile([1, GS], F32, tag="bi2")
            bh2_sb = cpool.tile([1, GS], F32, tag="bh2")
            outb_sb = cpool.tile([1, VS], F32, tag="outb")
            id_sb = cpool.tile([B, B], F32, tag="id")
            ones_sb = cpool.tile([1, B], F32, tag="ones")
            cbase_sb = cpool.tile([128, NCHUNK], F32, tag="cbase")

            for dst, src in ((wh1T_sb, wh1T), (wi2T_sb, wi2T),
                             (wh2T_sb, wh2T), (outwT_sb, outwT)):
                nc.sync.dma_start(
                    out=dst[:], in_=src.rearrange("(kt p) n -> p kt n", p=128))
            for dst, src in ((bh1_sb, bh1), (bi2_sb, bi2), (bh2_sb, bh2),
                             (outb_sb, outb), (id_sb, ident),
                             (cbase_sb, cbase)):
                nc.sync.dma_start(out=dst[:], in_=src)
            nc.vector.memset(ones_sb[:], 1.0)

            # ---------- state ----------
            hT = [spool.tile([128, KT, B], F32, tag=f"hT{l}", name=f"hT{l}")
                  for l in (0, 1)]
            hsl = [spool.tile([B, HS], F32, tag=f"hsl{l}", name=f"hsl{l}")
                   for l in (0, 1)]
            for l in (0, 1):
                nc.sync.dma_start(
                    out=hT[l][:], in_=h0T[l].rearrange("(kt p) b -> p kt b", p=128))
                nc.sync.dma_start(out=hsl[l][:], in_=h0s[l])
            idx16 = cpool.tile([16, 4], I16, tag="idx16")
            nc.vector.memset(idx16[:], 0)   # START_TOKEN = 0

            amax_out_prev = None

            for t in range(T):
                # ---- early (off critical path): gh matmuls from h(t-1) ----
                ps_gh1 = pg.tile([B, GS], F32, tag="gh1")
                for k in range(KT):
                    nc.tensor.matmul(ps_gh1[:], hT[0][:, k, :], wh1T_sb[:, k, :],
                                     start=(k == 0), stop=False)
                nc.tensor.matmul(ps_gh1[:], ones_sb[:], bh1_sb[:],
                                 start=False, stop=True)
                ps_gh2 = pg.tile([B, GS], F32, tag="gh2")
                for k in range(KT):
                    nc.tensor.matmul(ps_gh2[:], hT[1][:, k, :], wh2T_sb[:, k, :],
                                     start=(k == 0), stop=False)
                nc.tensor.matmul(ps_gh2[:], ones_sb[:], bh2_sb[:],
                                 start=False, stop=True)

                # ---- token combine from previous step's candidate AG ----
                if t > 0:
                    # 16 candidates per batch row: 8 cores x 2 col-group
                    # halves. AG block c is [128, 2] with rows h*64+j*16+p.
                    cand = wpool.tile([16, 16, 4, 2], F32, tag="cand")
                    nc.sync.dma_start(
                        out=cand[:],
                        in_=amax_out_prev[:].rearrange(
                            "c (h j p) v -> p (c h) j v", p=16, h=2))
                    maxv = wpool.tile([16, 4], F32, tag="maxv")
                    nc.vector.tensor_reduce(
                        out=maxv[:],
                        in_=cand[:, :, :, 0].rearrange("p ch j -> p j ch"),
                        axis=AX.X, op=ALU.max)
                    comb = wpool.tile([16, 4, 16], F32, tag="comb")
                    for j in range(4):
                        nc.vector.tensor_scalar(
                            out=comb[:, j, :], in0=cand[:, :, j, 0],
                            scalar1=maxv[:, j:j + 1], scalar2=-HUGE,
                            op0=ALU.subtract, op1=ALU.mult)
                        nc.vector.tensor_tensor(
                            out=comb[:, j, :], in0=comb[:, j, :],
                            in1=cand[:, :, j, 1], op=ALU.add)
                    winner = wpool.tile([16, 4], F32, tag="winner")
                    nc.vector.tensor_reduce(out=winner[:], in_=comb[:],
                                            axis=AX.X, op=ALU.min)
                    win_i = wpool.tile([16, 4], I32, tag="win_i")
                    nc.vector.tensor_copy(win_i[:], winner[:])
                    nc.vector.tensor_copy(idx16[:], win_i[:])

                # ---- layer-1 input gates via token gather ----
                gi1 = wpool.tile([128, 1, GS], F32, tag="gi1")
                nc.gpsimd.dma_gather(
                    out_ap=gi1[:], in_ap=g_tab, idxs_ap=idx16[:],
                    num_idxs=B, num_idxs_reg=B, elem_size=GS)

                # ---- GRU cells ----
                def cell(l, gi_ap, gh_ps):
                    rz_pre = wpool.tile([B, 256], F32, tag="rz_pre")
                    nc.vector.tensor_tensor(out=rz_pre[:], in0=gi_ap[:, 0:256],
                                            in1=gh_ps[:, 0:256], op=ALU.add)
                    rz = wpool.tile([B, 256], F32, tag="rz")
                    nc.scalar.activation(rz[:], rz_pre[:], ACTF.Sigmoid)
                    n_pre = wpool.tile([B, HS], F32, tag="n_pre")
                    nc.vector.tensor_tensor(out=n_pre[:], in0=rz[:, 0:128],
                                            in1=gh_ps[:, 256:384], op=ALU.mult)
                    nc.vector.tensor_tensor(out=n_pre[:], in0=n_pre[:],
                                            in1=gi_ap[:, 256:384], op=ALU.add)
                    n_t = wpool.tile([B, HS], F32, tag="n_t")
                    nc.scalar.activation(n_t[:], n_pre[:], ACTF.Tanh)
                    d_t = wpool.tile([B, HS], F32, tag="d_t")
                    nc.vector.tensor_tensor(out=d_t[:], in0=hsl[l][:],
                                            in1=n_t[:], op=ALU.subtract)
                    h_new = spool.tile([B, HS], F32, tag=f"hsl{l}", name=f"hnew{l}")
                    nc.vector.tensor_tensor(out=d_t[:], in0=rz[:, 128:256],
                                            in1=d_t[:], op=ALU.mult)
                    nc.vector.tensor_tensor(out=h_new[:], in0=d_t[:],
                                            in1=n_t[:], op=ALU.add)
                    return h_new

                def exchange(l, h_new):
                    # transpose [B, HS] -> [HS=128, B], AllGather to [H, B]
                    tr_ps = pg.tile([128, B], F32, tag="tr")
                    nc.tensor.transpose(tr_ps[:], h_new[:], id_sb[:])
                    hT_sl = wpool.tile([128, B], F32, tag="hT_sl")
                    nc.scalar.copy(hT_sl[:], tr_ps[:])
                    ag_in = dpool.tile([128, B], F32, tag=f"agin{l}", name=f"agin{l}")
                    ag_out = dpool.tile([NCORES, 128, B], F32, tag=f"agout{l}", name=f"agout{l}")
                    nc.sync.dma_start(out=ag_in[:], in_=hT_sl[:])
                    nc.gpsimd.collective_compute(
                        "AllGather", ALU.bypass, replica_groups=rg,
                        ins=[ag_in.opt()], outs=[ag_out.opt()])
                    hT_new = spool.tile([128, KT, B], F32, tag=f"hT{l}", name=f"hTn{l}")
                    nc.sync.dma_start(
                        out=hT_new[:], in_=ag_out[:].rearrange("r p b -> p r b"))
                    return hT_new

                h1_new = cell(0, gi1[0:B, 0, :], ps_gh1)
                hT1_new = exchange(0, h1_new)

                ps_gi2 = pg.tile([B, GS], F32, tag="gi2")
                for k in range(KT):
                    nc.tensor.matmul(ps_gi2[:], hT1_new[:, k, :], wi2T_sb[:, k, :],
                                     start=(k == 0), stop=False)
                nc.tensor.matmul(ps_gi2[:], ones_sb[:], bi2_sb[:],
                                 start=False, stop=True)

                gi2_sb = wpool.tile([B, GS], F32, tag="gi2_sb")
                nc.scalar.copy(gi2_sb[:], ps_gi2[:])
                h2_new = cell(1, gi2_sb, ps_gh2)
                hT2_new = exchange(1, h2_new)

                # ---- projection (vocab-sharded, col-tiled x2) ----
                logits_sb = wpool.tile([128, VH], F32, tag="logits")
                cv8 = wpool.tile([128, NCHUNK, 8], F32, tag="cv8")
                ci8 = wpool.tile([128, NCHUNK, 8], U32, tag="ci8")
                for c in range(NCHUNK):
                    ps_p = pp.tile([128, 512], F32, tag="proj")
                    lo = c * CW
                    for k in range(KT):
                        nc.tensor.matmul(
                            ps_p[0:B, 0:CW], hT2_new[:, k, :],
                            outwT_sb[:, k, lo:lo + CW],
                            start=(k == 0), stop=False, tile_position=(0, 0))
                        nc.tensor.matmul(
                            ps_p[B:128, 0:CW], hT2_new[:, k, :],
                            outwT_sb[:, k, VH + lo:VH + lo + CW],
                            start=(k == 0), stop=False, tile_position=(0, 64))
                    nc.tensor.matmul(ps_p[0:B, 0:CW], ones_sb[:],
                                     outb_sb[:, lo:lo + CW],
                                     start=False, stop=True, tile_position=(0, 0))
                    nc.tensor.matmul(ps_p[B:128, 0:CW], ones_sb[:],
                                     outb_sb[:, VH + lo:VH + lo + CW],
                                     start=False, stop=True, tile_position=(0, 64))
                    nc.scalar.copy(logits_sb[:, lo:lo + CW], ps_p[:, 0:CW])
                    nc.vector.max(cv8[:, c, :], logits_sb[:, lo:lo + CW])
                    nc.vector.max_index(ci8[:, c, :], cv8[:, c, :],
                                        logits_sb[:, lo:lo + CW])

                # logits out (two halves: psum rows 0:64 vocabA, 64:128 vocabB)
                nc.sync.dma_start(out=logits_out[t][:, 0:VH],
                                  in_=logits_sb[0:B, :])
                nc.sync.dma_start(out=logits_out[t][:, VH:VS],
                                  in_=logits_sb[B:128, :])

                # ---- local argmax combine ----
                gidx = wpool.tile([128, NCHUNK], F32, tag="gidx")
                nc.vector.tensor_copy(gidx[:], ci8[:, :, 0])
                nc.vector.tensor_tensor(out=gidx[:], in0=gidx[:],
                                        in1=cbase_sb[:], op=ALU.add)
                pack = wpool.tile([128, 2], F32, tag="pack")
                nc.vector.tensor_reduce(out=pack[:, 0:1], in_=cv8[:, :, 0],
                                        axis=AX.X, op=ALU.max)
                cmb = wpool.tile([128, NCHUNK], F32, tag="cmb")
                nc.vector.tensor_scalar(out=cmb[:], in0=cv8[:, :, 0],
                                        scalar1=pack[:, 0:1], scalar2=-HUGE,
                                        op0=ALU.subtract, op1=ALU.mult)
                nc.vector.tensor_tensor(out=cmb[:], in0=cmb[:], in1=gidx[:],
                                        op=ALU.add)
                nc.vector.tensor_reduce(out=pack[:, 1:2], in_=cmb[:],
                                        axis=AX.X, op=ALU.min)

                am_in = dpool.tile([128, 2], F32, tag="amin")
                am_out = dpool.tile([NCORES, 128, 2], F32, tag="amout")
                nc.sync.dma_start(out=am_in[:], in_=pack[:])
                nc.gpsimd.collective_compute(
                    "AllGather", ALU.bypass, replica_groups=rg,
                    ins=[am_in.opt()], outs=[am_out.opt()])
                amax_out_prev = am_out

                hT[0], hT[1] = hT1_new, hT2_new
                hsl[0], hsl[1] = h1_new, h2_new

            for l in (0, 1):
                nc.sync.dma_start(out=hfin[l], in_=hT[l][:])
    nc.compile()
    return nc


# ----------------------------------------------------------------------
# host side
# ----------------------------------------------------------------------

def prep(inputs):
    emb = np.asarray(inputs["emb"], np.float32)
    w_ih = np.asarray(inputs["w_ih"], np.float32)
    w_hh = np.asarray(inputs["w_hh"], np.float32)
    b_ih = np.asarray(inputs["b_ih"], np.float32)
    b_hh = np.asarray(inputs["b_hh"], np.float32)
    out_w = np.asarray(inputs["out_w"], np.float32)
    out_b = np.asarray(inputs["out_b"], np.float32)
    h0 = np.asarray(inputs["encoder_hidden"], np.float32)  # [2, B, H]

    emb_relu = np.maximum(emb, 0.0)
    G = emb_relu @ w_ih[0].T + b_ih[0]          # [V, 3H]
    outwT = np.ascontiguousarray(out_w.T)        # [H, V]
    h0T = np.ascontiguousarray(np.transpose(h0, (0, 2, 1)))  # [2, H, B]
    ident = np.eye(B, dtype=np.float32)

    in_maps = []
    for c in range(NCORES):
        gs = np.r_[c * HS:(c + 1) * HS,
                   H + c * HS:H + (c + 1) * HS,
                   2 * H + c * HS:2 * H + (c + 1) * HS]
        cbase = np.zeros((128, NCHUNK), np.float32)
        cbase[:, :] = (np.arange(NCHUNK) * CW)[None, :]
        cbase[B:, :] += VH
        cbase += c * VS
        in_maps.append({
            "g_tab": np.ascontiguousarray(G[:, gs]),
            "wh1T": np.ascontiguousarray(w_hh[0].T[:, gs]),
            "wi2T": np.ascontiguousarray(w_ih[1].T[:, gs]),
            "wh2T": np.ascontiguousarray(w_hh[1].T[:, gs]),
            "bh1": np.ascontiguousarray(b_hh[0][gs][None, :]),
            "bi2": np.ascontiguousarray(b_ih[1][gs][None, :]),
            "bh2": np.ascontiguousarray(b_hh[1][gs][None, :]),
            "outwT": np.ascontiguousarray(outwT[:, c * VS:(c + 1) * VS]),
            "outb": np.ascontiguousarray(out_b[c * VS:(c + 1) * VS][None, :]),
            "h0T": h0T,
            "h0s": np.ascontiguousarray(h0[:, :, c * HS:(c + 1) * HS]),
            "ident": ident,
            "cbase": cbase,
        })
    return in_maps


_CACHE = {}


def run(inputs, trace=False):
    T = int(np.asarray(inputs.get("max_length", 32)))
    if T not in _CACHE:
        _CACHE[T] = build(T)
    nc = _CACHE[T]
    in_maps = prep(inputs)
    res = run_bass_kernel_spmd(nc, in_maps, core_ids=list(range(NCORES)),
                               trace=trace)
    # assemble full outputs
    logits = np.concatenate(
        [res.results[c]["logits_out"] for c in range(NCORES)], axis=2)
    decoder_outputs = np.ascontiguousarray(np.transpose(logits, (1, 0, 2)))
    hf = res.results[0]["hfin"]  # [2, 128, KT, B]
    h_final = np.ascontiguousarray(
        np.transpose(hf.reshape(2, 128, KT, B), (0, 3, 2, 1))
        .reshape(2, B, H))
    return (decoder_outputs, h_final), res


def kernel(**inputs):
    (decoder_outputs, h_final), _ = run(inputs, trace=False)
    return decoder_outputs, h_final
